# revision 27
# baseline (speedup 1.0000x reference)
"""Trainium2 Bass kernel for nn_Eq1to3 (eset_ops_1_to_3 + einsum broadcast expansion).

Reference computation (N=16, D=64, S=32, M=48, BASIS=4):
    t[b,n,s,m] = sum_d coefs[d,s,b] * x[n,d,m]        # tiny einsum
    out[n,s,i,j,k] = t0[n,s,i] + t1[n,s,j] + t2[n,s,k]
                     + (i==j==k) * t3[n,s,i] + bias[s]
Full output (16, 32, 48, 48, 48) f32 = 226.5 MB; device emits bf16 (host
upcasts after gather; rel err ~5e-3 << 2e-2 tolerance). 14.16 MB/core of
HBM writes => ~34 us stream floor at the measured ~420 GB/s/core; the
kernel is output-DMA-bound, so everything else hides behind the stream.

Sharding: data-parallel over N across 8 cores (2 batches/core). Per core the
output is [3072 rows p=(n',s,i), 2304 cols (j,k)].

Partition map (l-major): q = 64*l + qq with qq = n'*S + s in [0,64) and
l = q//64; partition q holds i(q,u) = 24*l + u for u in [0,24). The device
DRAM tensor y stores row 24*q + u (so every partition owns one contiguous
110 KB span and every DMA AP is uniform); the host gather transposes the
(l, qq) split back to (qq, l) order -- pure reshape/transpose.

Key structure: every row of a given partition is the SAME per-partition plane
    PL[q, (j,k)] = t1[ns(q), j] + t2[ns(q), k] + bias[s(q)]
shifted by the per-row scalar t0[ns(q), i(q,u)], plus one superdiagonal cell
(j,k)=(i,i) worth t3[n,s,i]. So:
  - PE builds tiny vectors T1[q,j], T2b[q,k]=t2+bias, T0[q,u], T3[q,u] via
    indicator-weight matmuls (l-masked weights fold (n',l) into K=128; every
    matmul <= 1 PSUM bank -- the ISA caps matmul free size at 512 f32).
  - One DVE tensor_tensor add builds PL in SBUF bf16 (t1 read from PSUM).
  - DVE emits each row with ONE tensor_scalar_add (PL bf16 SBUF step-1 ->
    4x perf mode, ~0.78us/row; the per-partition scalar T0 is exempt from
    the 2x dtype rules) into a full-output SBUF buffer (110.6 KB/partition,
    no recycling), then adds each chunk's diagonal cells with two tiny
    strided adds (cell offset u*(2304+49) + 1176*l is partition-uniform per
    l-half thanks to the l-major map).
  - GpSimd does nothing (its ops stall concurrently-running 2-port DVE ops
    on the shared SBUF port, and SWDGE aggravates the DMA-engine-15
    straggler).
  - Output DMA alternates the two HWDGE rings (SP / ACT). Rows 0-1 ship as
    half-rows to start the HBM stream early; later rows in 2- then 4-row
    chunks. Once started the stream runs gapless at ~420 GB/s; kernel time
    ~= first-DMA-byte (~14 us: ~7 us fixed preamble + input load/receipt +
    PL build) + 34 us stream + ~2.7 us fixed event-semaphore epilogue.
"""

import numpy as np

N, D, S, M, BASIS = 16, 64, 32, 48, 4
N_CORES = 8
NL = N // N_CORES              # batches per core (2)
NS = NL * S                    # (n',s) groups per core (64)
ROWS = NS * M                  # output rows per core (3072)
JK = M * M                     # free size per row (2304)
P = 128                        # partitions
HALF = M // 2                  # rows per partition (24)
K = NL * D                     # contraction size (128)
XW1_COLS = M + 3 * P           # x2 | w1 | w2 | wb     (gates T1/T2 -> PL)
XW2_COLS = 4 * P               # w0l0 | w0l1 | w3l0 | w3l1  (gates T0/T3)

# output chunks in u-rows after rows 0-1 (which ship as half-rows)
CHUNKS = [2, 2, 2, 4, 4, 4, 4]
HJ = JK // 2                   # half-row free size (1152)

_PROG = None


def _build_prog():
    import concourse.bacc as bacc
    import concourse.tile as tile
    import concourse.mybir as mybir

    f32 = mybir.dt.float32
    bf16 = mybir.dt.bfloat16
    COPY = mybir.ActivationFunctionType.Copy
    nc = bacc.Bacc("TRN2", target_bir_lowering=False, debug=False,
                   num_devices=N_CORES)

    xw1_d = nc.dram_tensor("xw1", [K, XW1_COLS], bf16,
                           kind="ExternalInput").ap()
    xw2_d = nc.dram_tensor("xw2", [K, XW2_COLS], bf16,
                           kind="ExternalInput").ap()
    y_d = nc.dram_tensor("y", [ROWS, JK], bf16, kind="ExternalOutput").ap()

    with tile.TileContext(nc) as tc:
        with (
            tc.tile_pool(name="const", bufs=1) as cpool,
            tc.tile_pool(name="psum", bufs=1, space="PSUM") as ppool,
        ):
            # ---- inputs: two parallel loads (SP + ACT rings) so the
            # PL-critical columns pay the ~2us completion receipt once,
            # on a smaller transfer ----
            xw1_sb = cpool.tile([K, XW1_COLS], bf16)
            nc.sync.dma_start(out=xw1_sb[:], in_=xw1_d[:])
            xw2_sb = cpool.tile([K, XW2_COLS], bf16)
            nc.scalar.dma_start(out=xw2_sb[:], in_=xw2_d[:])
            ones_sb = cpool.tile([1, 1], bf16)
            nc.vector.memset(ones_sb[:], 1.0)

            x2_sb = xw1_sb[:, 0:M]
            w1_l = lambda: xw1_sb[:, M + 0 * P:M + 1 * P]
            w2_l = lambda: xw1_sb[:, M + 1 * P:M + 2 * P]
            wb_l = lambda: xw1_sb[:1, M + 2 * P:M + 3 * P]
            w0_l = lambda li: xw2_sb[:, (0 + li) * P:(1 + li) * P]
            w3_l = lambda li: xw2_sb[:, (2 + li) * P:(3 + li) * P]

            # ---- tiny per-(n,s) vectors via PE (every matmul <= 1 PSUM
            # bank): T1, T2b = t2 + bias, T0, T3 ----
            T1_ps = ppool.tile([P, M], f32)
            nc.tensor.matmul(T1_ps[:], w1_l(), x2_sb[:], start=True, stop=True)
            T2_ps = ppool.tile([P, M], f32)
            nc.tensor.matmul(T2_ps[:], w2_l(), x2_sb[:], start=True,
                             stop=False)
            rhs_b = ones_sb[0:1, 0:1].broadcast_to((1, M))
            nc.tensor.matmul(T2_ps[:], wb_l(), rhs_b, start=False, stop=True)
            T0_ps = ppool.tile([P, HALF], f32)
            T3_ps = ppool.tile([P, HALF], f32)
            for li in range(2):
                nc.tensor.matmul(T0_ps[:], w0_l(li),
                                 x2_sb[:, HALF * li:HALF * (li + 1)],
                                 start=(li == 0), stop=(li == 1))
            for li in range(2):
                nc.tensor.matmul(T3_ps[:], w3_l(li),
                                 x2_sb[:, HALF * li:HALF * (li + 1)],
                                 start=(li == 0), stop=(li == 1))
            T2G = cpool.tile([P, M], f32)
            nc.scalar.activation(T2G[:], T2_ps[:], COPY)
            T0G = cpool.tile([P, HALF], f32)
            nc.scalar.activation(T0G[:], T0_ps[:], COPY)
            T3G = cpool.tile([P, HALF], f32)
            nc.scalar.activation(T3G[:], T3_ps[:], COPY)

            # ---- PL[q, (j,k)] = t1[j] + t2[k] + bias: one DVE TT add
            # (t1 read straight from PSUM -- skips a mirror copy; the TT is
            # 1x-mode regardless because in_j is a broadcast AP) ----
            PL_sb = cpool.tile([P, JK], bf16)
            PL3 = PL_sb.rearrange("q (j k) -> q j k", k=M)
            in_j = T1_ps[:, :, None].broadcast_to((P, M, M))
            in_k = T2G[:, None, :].broadcast_to((P, M, M))
            nc.vector.tensor_add(out=PL3, in0=in_j, in1=in_k)

            # ---- full per-core output lives in SBUF; DMA never waits on
            # buffer recycling ----
            out_sb = cpool.tile([P, HALF * JK], bf16)
            out_v = out_sb.rearrange("q (u f) -> q u f", u=HALF)

            # device DRAM layout is l-major: row 24*q + u for partition q
            # (the host gather transposes (l, qq) back to (qq, l) order)
            y_v = y_d.rearrange("(q u) f -> q u f", q=P)

            # rows 0-1 ship as half-rows (smallest lead-in that keeps the
            # stream gapless); their diag cells go on the idle ACT engine --
            # a dependent DVE op would slip one slot behind the next ready
            # tensor_scalar in the Tile scheduler, delaying the first DMA.
            # Cell for (u, l) sits at in-row offset (24*l + u)*49: half l.
            IDENT = mybir.ActivationFunctionType.Identity
            for u in range(2):
                for h in range(2):
                    nc.vector.tensor_scalar_add(
                        out_v[:, u, HJ * h:HJ * (h + 1)],
                        PL_sb[:, HJ * h:HJ * (h + 1)], T0G[:, u:u + 1])
                    off = u * (JK + M + 1) + HALF * (M + 1) * h
                    cell = out_sb[64 * h:64 * (h + 1), off:off + 1]
                    nc.scalar.activation(cell, cell, IDENT,
                                         bias=T3G[64 * h:64 * (h + 1), u:u + 1])
                    eng = nc.sync if (2 * u + h) % 2 == 0 else nc.scalar
                    eng.dma_start(out=y_v[:, u, HJ * h:HJ * (h + 1)],
                                  in_=out_v[:, u, HJ * h:HJ * (h + 1)])

            u0 = 2
            for c, cw in enumerate(CHUNKS):
                for u in range(u0, u0 + cw):
                    nc.vector.tensor_scalar_add(
                        out_v[:, u], PL_sb[:], T0G[:, u:u + 1])
                # superdiagonal cells: offset(u; l) = u*(JK + M+1) + HALF*(M+1)*l
                for li in range(2):
                    dv = out_sb[64 * li:64 * (li + 1),
                                u0 * (JK + M + 1) + HALF * (M + 1) * li::JK + M + 1]
                    dv = dv[:, :cw]
                    nc.vector.tensor_add(
                        out=dv, in0=dv,
                        in1=T3G[64 * li:64 * (li + 1), u0:u0 + cw])
                eng = nc.sync if c % 2 == 0 else nc.scalar
                eng.dma_start(out=y_v[:, u0:u0 + cw, :],
                              in_=out_v[:, u0:u0 + cw, :])
                u0 += cw

    nc.compile()
    return nc


def _get_prog():
    global _PROG
    if _PROG is None:
        _PROG = _build_prog()
    return _PROG


def _make_in_maps(x, coefs, bias):
    import ml_dtypes

    bf = ml_dtypes.bfloat16
    x = np.asarray(x, dtype=np.float32)
    coefs = np.asarray(coefs, dtype=np.float32)
    bias = np.asarray(bias, dtype=np.float32)

    # partition q = 64*l + qq:  l = q//64, qq = q%64 = n'*S + s
    q = np.arange(P)
    l_of = q // NS
    n_of = (q % NS) // S
    s_of = (q % NS) % S
    nd_n = np.repeat(np.arange(NL), D)                # (K,) n' of row
    nd_d = np.tile(np.arange(D), NL)                  # (K,) d of row
    sel = (nd_n[:, None] == n_of[None, :]).astype(np.float32)  # (K, P)

    def w_of(b):
        return coefs[nd_d[:, None], s_of[None, :], b] * sel

    lmask = [(l_of == li).astype(np.float32)[None, :] for li in range(2)]
    bias_row = np.zeros((K, P), np.float32)
    bias_row[0] = bias.reshape(S)[s_of]

    w0 = w_of(0)
    w3 = w_of(3)
    xw2 = np.ascontiguousarray(
        np.concatenate([w0 * lmask[0], w0 * lmask[1],
                        w3 * lmask[0], w3 * lmask[1]], axis=1).astype(bf))

    in_maps = []
    for core in range(N_CORES):
        x2 = x[NL * core:NL * (core + 1)].reshape(NL * D, M)
        xw1 = np.ascontiguousarray(
            np.concatenate([x2, w_of(1), w_of(2), bias_row],
                           axis=1).astype(bf))
        in_maps.append({"xw1": xw1, "xw2": xw2})
    return in_maps


def run(x, coefs, bias, **run_kwargs):
    """Run on hardware; returns (full_output, BassKernelResults)."""
    from concourse.bass_utils import run_bass_kernel_spmd

    prog = _get_prog()
    in_maps = _make_in_maps(x, coefs, bias)
    res = run_bass_kernel_spmd(prog, in_maps, list(range(N_CORES)), **run_kwargs)
    # device row order is (l, qq, u); output order needs (qq, l, u) = (n',s,i)
    out = np.concatenate(
        [np.asarray(res.results[i]["y"]).astype(np.float32)
         .reshape(2, NS, HALF, JK).transpose(1, 0, 2, 3)
         .reshape(NL, S, M, M, M) for i in range(N_CORES)],
        axis=0)
    return out, res


def kernel(x, coefs, bias):
    out, _ = run(x, coefs, bias)
    return out


# revision 29
# speedup vs baseline: 1.0482x; 1.0482x over previous
"""Trainium2 Bass kernel for nn_Eq1to3 (eset_ops_1_to_3 + einsum broadcast expansion).

Reference computation (N=16, D=64, S=32, M=48, BASIS=4):
    t[b,n,s,m] = sum_d coefs[d,s,b] * x[n,d,m]        # tiny einsum
    out[n,s,i,j,k] = t0[n,s,i] + t1[n,s,j] + t2[n,s,k]
                     + (i==j==k) * t3[n,s,i] + bias[s]
Full output (16, 32, 48, 48, 48) f32 = 226.5 MB; device emits bf16 (host
upcasts after gather; rel err ~5e-3 << 2e-2 tolerance). 14.16 MB/core of
HBM writes => ~34 us stream floor at the measured ~420 GB/s/core; the
kernel is output-DMA-bound, so everything else hides behind the stream.

Sharding: data-parallel over N across 8 cores (2 batches/core). Per core the
output is [3072 rows p=(n',s,i), 2304 cols (j,k)].

Partition map (l-major): q = 64*l + qq with qq = n'*S + s in [0,64) and
l = q//64; partition q holds i(q,u) = 24*l + u for u in [0,24). The device
DRAM tensor y stores row 24*q + u (so every partition owns one contiguous
110 KB span and every DMA AP is uniform); the host gather transposes the
(l, qq) split back to (qq, l) order -- pure reshape/transpose.

Key structure: every row of a given partition is the SAME per-partition plane
    PL[q, (j,k)] = t1[ns(q), j] + t2[ns(q), k] + bias[s(q)]
shifted by the per-row scalar t0[ns(q), i(q,u)], plus one superdiagonal cell
(j,k)=(i,i) worth t3[n,s,i]. So:
  - PE builds tiny vectors T1[q,j], T2b[q,k]=t2+bias, T0[q,u], T3[q,u] via
    indicator-weight matmuls (l-masked weights fold (n',l) into K=128; every
    matmul <= 1 PSUM bank -- the ISA caps matmul free size at 512 f32).
  - One DVE tensor_tensor add builds PL in SBUF bf16 (t1 read from PSUM).
  - DVE emits each row with ONE tensor_scalar_add (PL bf16 SBUF step-1 ->
    4x perf mode, ~0.78us/row; the per-partition scalar T0 is exempt from
    the 2x dtype rules) into a full-output SBUF buffer (110.6 KB/partition,
    no recycling), then adds each chunk's diagonal cells with two tiny
    strided adds (cell offset u*(2304+49) + 1176*l is partition-uniform per
    l-half thanks to the l-major map).
  - GpSimd does nothing (its ops stall concurrently-running 2-port DVE ops
    on the shared SBUF port, and SWDGE aggravates the DMA-engine-15
    straggler).
  - Output DMA alternates the two HWDGE rings (SP / ACT). Rows 0-1 ship as
    half-rows to start the HBM stream early; later rows in 2- then 4-row
    chunks. Once started the stream runs gapless at ~420 GB/s; kernel time
    ~= first-DMA-byte (~14 us: ~7 us fixed preamble + input load/receipt +
    PL build) + 34 us stream + ~2.7 us fixed event-semaphore epilogue.
"""

import numpy as np

N, D, S, M, BASIS = 16, 64, 32, 48, 4
N_CORES = 8
NL = N // N_CORES              # batches per core (2)
NS = NL * S                    # (n',s) groups per core (64)
ROWS = NS * M                  # output rows per core (3072)
JK = M * M                     # free size per row (2304)
P = 128                        # partitions
HALF = M // 2                  # rows per partition (24)
K = NL * D                     # contraction size (128)
XW1_COLS = M + 3 * P           # x2 | w1 | w2 | wb     (gates T1/T2 -> PL)
XW2_COLS = 4 * P               # w0l0 | w0l1 | w3l0 | w3l1  (gates T0/T3)

# output chunks in u-rows after rows 0-1 (which ship as half-rows)
CHUNKS = [2, 2, 2, 4, 4, 4, 4]
HJ = JK // 2                   # half-row free size (1152)

_PROG = None


def _build_prog():
    import concourse.bacc as bacc
    import concourse.tile as tile
    import concourse.mybir as mybir

    f32 = mybir.dt.float32
    bf16 = mybir.dt.bfloat16
    COPY = mybir.ActivationFunctionType.Copy
    nc = bacc.Bacc("TRN2", target_bir_lowering=False, debug=False,
                   num_devices=N_CORES)

    xw1_d = nc.dram_tensor("xw1", [K, XW1_COLS], bf16,
                           kind="ExternalInput").ap()
    xw2_d = nc.dram_tensor("xw2", [K, XW2_COLS], bf16,
                           kind="ExternalInput").ap()
    y_d = nc.dram_tensor("y", [ROWS, JK], bf16, kind="ExternalOutput").ap()

    with tile.TileContext(nc) as tc:
        with (
            tc.tile_pool(name="const", bufs=1) as cpool,
            tc.tile_pool(name="psum", bufs=1, space="PSUM") as ppool,
        ):
            # ---- inputs: two parallel loads. The PL-critical columns go on
            # the ACT ring: ACT's sequencer exits the preamble ~1.2us before
            # Sync (whose preamble ends in a 0.7us DRAIN), and the ACT table
            # load is inserted before the first activation, i.e. after this
            # issue ----
            xw1_sb = cpool.tile([K, XW1_COLS], bf16)
            nc.scalar.dma_start(out=xw1_sb[:], in_=xw1_d[:])
            xw2_sb = cpool.tile([K, XW2_COLS], bf16)
            nc.sync.dma_start(out=xw2_sb[:], in_=xw2_d[:])
            ones_sb = cpool.tile([1, 1], bf16)
            nc.vector.memset(ones_sb[:], 1.0)

            x2_sb = xw1_sb[:, 0:M]
            w1_l = lambda: xw1_sb[:, M + 0 * P:M + 1 * P]
            w2_l = lambda: xw1_sb[:, M + 1 * P:M + 2 * P]
            wb_l = lambda: xw1_sb[:1, M + 2 * P:M + 3 * P]
            w0_l = lambda li: xw2_sb[:, (0 + li) * P:(1 + li) * P]
            w3_l = lambda li: xw2_sb[:, (2 + li) * P:(3 + li) * P]

            # ---- tiny per-(n,s) vectors via PE (every matmul <= 1 PSUM
            # bank): T1, T2b = t2 + bias, T0, T3 ----
            T1_ps = ppool.tile([P, M], f32)
            nc.tensor.matmul(T1_ps[:], w1_l(), x2_sb[:], start=True, stop=True)
            T2_ps = ppool.tile([P, M], f32)
            nc.tensor.matmul(T2_ps[:], w2_l(), x2_sb[:], start=True,
                             stop=False)
            rhs_b = ones_sb[0:1, 0:1].broadcast_to((1, M))
            nc.tensor.matmul(T2_ps[:], wb_l(), rhs_b, start=False, stop=True)
            T0_ps = ppool.tile([P, HALF], f32)
            T3_ps = ppool.tile([P, HALF], f32)
            for li in range(2):
                nc.tensor.matmul(T0_ps[:], w0_l(li),
                                 x2_sb[:, HALF * li:HALF * (li + 1)],
                                 start=(li == 0), stop=(li == 1))
            for li in range(2):
                nc.tensor.matmul(T3_ps[:], w3_l(li),
                                 x2_sb[:, HALF * li:HALF * (li + 1)],
                                 start=(li == 0), stop=(li == 1))
            T2G = cpool.tile([P, M], f32)
            nc.scalar.activation(T2G[:], T2_ps[:], COPY)
            T0G = cpool.tile([P, HALF], f32)
            nc.scalar.activation(T0G[:], T0_ps[:], COPY)
            T3G = cpool.tile([P, HALF], f32)
            nc.scalar.activation(T3G[:], T3_ps[:], COPY)

            # ---- PL[q, (j,k)] = t1[j] + t2[k] + bias: one DVE TT add
            # (t1 read straight from PSUM -- skips a mirror copy; the TT is
            # 1x-mode regardless because in_j is a broadcast AP) ----
            PL_sb = cpool.tile([P, JK], bf16)
            PL3 = PL_sb.rearrange("q (j k) -> q j k", k=M)
            in_j = T1_ps[:, :, None].broadcast_to((P, M, M))
            in_k = T2G[:, None, :].broadcast_to((P, M, M))
            nc.vector.tensor_add(out=PL3, in0=in_j, in1=in_k)

            # ---- full per-core output lives in SBUF; DMA never waits on
            # buffer recycling ----
            out_sb = cpool.tile([P, HALF * JK], bf16)
            out_v = out_sb.rearrange("q (u f) -> q u f", u=HALF)

            # device DRAM layout is l-major: row 24*q + u for partition q
            # (the host gather transposes (l, qq) back to (qq, l) order)
            y_v = y_d.rearrange("(q u) f -> q u f", q=P)

            # row 0 ships as four quarter-rows, DIAG-FREE quarters first:
            # the diag cells sit at in-row offsets 0 (l=0) and 1274 (l=1),
            # i.e. in quarters 0 and 2 -- so quarters 1 and 3 gate only on
            # their tensor_scalar and the very first DMA skips the diag
            # dependency chain. Diag cells go on the idle ACT engine (a
            # dependent DVE op would slip one slot behind the next ready
            # tensor_scalar in the Tile scheduler).
            IDENT = mybir.ActivationFunctionType.Identity
            QF = JK // 4
            for n, h in enumerate((1, 3, 0, 2)):
                nc.vector.tensor_scalar_add(
                    out_v[:, 0, QF * h:QF * (h + 1)],
                    PL_sb[:, QF * h:QF * (h + 1)], T0G[:, 0:1])
                if h in (0, 2):
                    li = h // 2
                    off = HALF * (M + 1) * li
                    cell = out_sb[64 * li:64 * (li + 1), off:off + 1]
                    nc.scalar.activation(cell, cell, IDENT,
                                         bias=T3G[64 * li:64 * (li + 1), 0:1])
                eng = nc.sync if n % 2 == 0 else nc.scalar
                eng.dma_start(out=y_v[:, 0, QF * h:QF * (h + 1)],
                              in_=out_v[:, 0, QF * h:QF * (h + 1)])

            # row 1 as half-rows (its cells sit at in-row 49 / 1323: one in
            # each half, so halves gate on one ACT diag each)
            for h in range(2):
                nc.vector.tensor_scalar_add(
                    out_v[:, 1, HJ * h:HJ * (h + 1)],
                    PL_sb[:, HJ * h:HJ * (h + 1)], T0G[:, 1:2])
                off = (JK + M + 1) + HALF * (M + 1) * h
                cell = out_sb[64 * h:64 * (h + 1), off:off + 1]
                nc.scalar.activation(cell, cell, IDENT,
                                     bias=T3G[64 * h:64 * (h + 1), 1:2])
                eng = nc.sync if h == 0 else nc.scalar
                eng.dma_start(out=y_v[:, 1, HJ * h:HJ * (h + 1)],
                              in_=out_v[:, 1, HJ * h:HJ * (h + 1)])

            u0 = 2
            for c, cw in enumerate(CHUNKS):
                for u in range(u0, u0 + cw):
                    nc.vector.tensor_scalar_add(
                        out_v[:, u], PL_sb[:], T0G[:, u:u + 1])
                # superdiagonal cells: offset(u; l) = u*(JK + M+1) + HALF*(M+1)*l
                for li in range(2):
                    dv = out_sb[64 * li:64 * (li + 1),
                                u0 * (JK + M + 1) + HALF * (M + 1) * li::JK + M + 1]
                    dv = dv[:, :cw]
                    nc.vector.tensor_add(
                        out=dv, in0=dv,
                        in1=T3G[64 * li:64 * (li + 1), u0:u0 + cw])
                eng = nc.sync if c % 2 == 0 else nc.scalar
                eng.dma_start(out=y_v[:, u0:u0 + cw, :],
                              in_=out_v[:, u0:u0 + cw, :])
                u0 += cw

    nc.compile()
    return nc


def _get_prog():
    global _PROG
    if _PROG is None:
        _PROG = _build_prog()
    return _PROG


def _make_in_maps(x, coefs, bias):
    import ml_dtypes

    bf = ml_dtypes.bfloat16
    x = np.asarray(x, dtype=np.float32)
    coefs = np.asarray(coefs, dtype=np.float32)
    bias = np.asarray(bias, dtype=np.float32)

    # partition q = 64*l + qq:  l = q//64, qq = q%64 = n'*S + s
    q = np.arange(P)
    l_of = q // NS
    n_of = (q % NS) // S
    s_of = (q % NS) % S
    nd_n = np.repeat(np.arange(NL), D)                # (K,) n' of row
    nd_d = np.tile(np.arange(D), NL)                  # (K,) d of row
    sel = (nd_n[:, None] == n_of[None, :]).astype(np.float32)  # (K, P)

    def w_of(b):
        return coefs[nd_d[:, None], s_of[None, :], b] * sel

    lmask = [(l_of == li).astype(np.float32)[None, :] for li in range(2)]
    bias_row = np.zeros((K, P), np.float32)
    bias_row[0] = bias.reshape(S)[s_of]

    w0 = w_of(0)
    w3 = w_of(3)
    xw2 = np.ascontiguousarray(
        np.concatenate([w0 * lmask[0], w0 * lmask[1],
                        w3 * lmask[0], w3 * lmask[1]], axis=1).astype(bf))

    in_maps = []
    for core in range(N_CORES):
        x2 = x[NL * core:NL * (core + 1)].reshape(NL * D, M)
        xw1 = np.ascontiguousarray(
            np.concatenate([x2, w_of(1), w_of(2), bias_row],
                           axis=1).astype(bf))
        in_maps.append({"xw1": xw1, "xw2": xw2})
    return in_maps


def run(x, coefs, bias, **run_kwargs):
    """Run on hardware; returns (full_output, BassKernelResults)."""
    from concourse.bass_utils import run_bass_kernel_spmd

    prog = _get_prog()
    in_maps = _make_in_maps(x, coefs, bias)
    res = run_bass_kernel_spmd(prog, in_maps, list(range(N_CORES)), **run_kwargs)
    # device row order is (l, qq, u); output order needs (qq, l, u) = (n',s,i)
    out = np.concatenate(
        [np.asarray(res.results[i]["y"]).astype(np.float32)
         .reshape(2, NS, HALF, JK).transpose(1, 0, 2, 3)
         .reshape(NL, S, M, M, M) for i in range(N_CORES)],
        axis=0)
    return out, res


def kernel(x, coefs, bias):
    out, _ = run(x, coefs, bias)
    return out
